# revision 52
# baseline (speedup 1.0000x reference)
"""Trainium2 Bass kernel for nn_Att_23313082483285 (GNN message passing).

Strategy: partition agent nodes across 8 cores (8192 each). Host routes each
edge to the core owning its destination agent (hi), groups edges by 128-node
block, splits each block's edges by wi < 32768 (dma_gather idx is int16), and
pads each (block, half) to a fixed tile count so all cores run one SPMD
program.

v2 redesign (vs baseline): GroupNorm means come from an augmented 129th
weight column (mean is linear in the matmul output); variance via one fused
scalar_tensor_tensor (x-m)*x with accum_out per tile (4x DVE mode on bf16
SBUF); GN apply via tensor_scalar with two per-partition scalar pointers
(sub mean, mult rinv) at 4x; ReLU folded into the post-transpose PSUM->SBUF
copy or the apply's max op. One-hot scatter masks are host-precomputed and
DMA'd. The per-edge @ctx_w2 matmul is reassociated past the scatter-sum:
scatter relu'd g directly per tile, apply ctx_w2 once per 128-node block.
agts@agt_w accumulates directly into the block PSUM. The GroupNorm before
lin_w needs no variance at all (per-row scale invariance of the next GN).
"""
import sys
sys.path.insert(0, '/opt/trn_rl_repo')

import numpy as np
import ml_dtypes
from contextlib import ExitStack

from concourse import bass, mybir, tile
import concourse.bacc as bacc
from concourse.bass_utils import run_bass_kernel_spmd
from concourse.masks import make_identity

bf16 = ml_dtypes.bfloat16
P = 128
N_AGT = 65536
N_CTX = 65536
E = 400000
D = 128
DA = 129                       # D + mean column
EPS = 1e-5
NCORES = 8
NPC = N_AGT // NCORES          # 8192 nodes per core
NBLK = NPC // P                # 64 blocks per core
CTX_HALF = 32768
G_TILES = 32                   # tiles per dma_gather op (4096 idxs)

f32 = mybir.dt.float32
bft = mybir.dt.bfloat16
i16 = mybir.dt.int16
AF = mybir.ActivationFunctionType
ALU = mybir.AluOpType


def _wrap16(flat_idx):
    """dma_gather idx layout: [16, n/16] with idx[c,k]=flat[k*16+c], tiled x8."""
    w = flat_idx.reshape(-1, 16).T.astype(np.int16)
    return np.ascontiguousarray(np.tile(w, (8, 1)))


def _host_prep(agts, ctx, agt_ctrs, ctx_ctrs, hi, wi, weights):
    """Route/pad edges per core; build all per-core device input arrays."""
    hi = np.asarray(hi).astype(np.int64)
    wi = np.asarray(wi).astype(np.int64)
    agts = np.asarray(agts, dtype=np.float32)
    ctx = np.asarray(ctx, dtype=np.float32)
    d_all = (np.asarray(agt_ctrs, np.float32)[hi]
             - np.asarray(ctx_ctrs, np.float32)[wi])          # [E, 2]

    blk_global = hi // P          # 0..511
    lidx_all = hi % P
    is_hi = wi >= CTX_HALF

    # per (global block, half) edge lists
    lists = [[[] for _ in range(2)] for _ in range(N_AGT // P)]
    order = np.argsort(blk_global * 2 + is_hi, kind='stable')
    for e in order:
        lists[blk_global[e]][1 if is_hi[e] else 0].append(e)

    t_lo = max(max((len(l[0]) for l in lists), default=0), 1)
    t_hi = max(max((len(l[1]) for l in lists), default=0), 1)
    T_LO = -(-(-(-t_lo // P)) // 4) * 4   # round tiles up to multiple of 4
    T_HI = -(-(-(-t_hi // P)) // 4) * 4
    T_B = T_LO + T_HI
    NT = NBLK * T_B               # tiles per core
    EPAD = NT * P

    cores = []
    for k in range(NCORES):
        dT4 = np.zeros((4, EPAD), np.float32)
        mask = np.zeros((P, NT * P), np.float32)   # one-hot scatter masks
        qflat = np.zeros(EPAD, np.int64)
        lo_flat = np.zeros(NBLK * T_LO * P, np.int64)
        hi_flat = np.zeros(NBLK * T_HI * P, np.int64)

        for b in range(NBLK):
            gb = k * NBLK + b
            for half, (TH, flat) in enumerate(
                    ((T_LO, lo_flat), (T_HI, hi_flat))):
                edges = lists[gb][half]
                assert len(edges) <= TH * P, (
                    f"block overflow core {k} blk {b} half {half}: "
                    f"{len(edges)} > {TH * P}")
                for j, e in enumerate(edges):
                    tl = j // P          # tile within this half
                    p = j % P
                    t = b * T_B + (0 if half == 0 else T_LO) + tl  # global tile
                    col = t * P + p
                    dT4[0, col] = d_all[e, 0]
                    dT4[1, col] = d_all[e, 1]
                    dT4[2, col] = 1.0
                    mask[p, t * P + lidx_all[e]] = 1.0
                    qflat[col] = lidx_all[e] + (blk_global[e] % NBLK) * P
                    si = b * TH * P + tl * P + p   # slot in half-stream
                    flat[si] = wi[e] if half == 0 else wi[e] - CTX_HALF

        cores.append(dict(
            agtsT=np.ascontiguousarray(agts[k * NPC:(k + 1) * NPC].T.astype(bf16)),
            agts_res=np.ascontiguousarray(agts[k * NPC:(k + 1) * NPC]),
            dT4=dT4.astype(bf16),
            maskh=mask.astype(bf16),
            qidx=_wrap16(qflat),
            widx_lo=_wrap16(lo_flat),
            widx_hi=_wrap16(hi_flat),
        ))

    shared = dict(ctx_bf=np.ascontiguousarray(ctx.astype(bf16)), **weights)
    return cores, shared, T_LO, T_HI


def _build_program(T_LO, T_HI):
    T_B = T_LO + T_HI
    NT = NBLK * T_B
    NST = NT // 4                 # super-tiles of 4 tiles
    N_LO = NBLK * T_LO            # lo tiles per core
    N_HI = NBLK * T_HI
    ST_PER_BLK = T_B // 4         # STs per block (2 when T_B=8)
    INV_D = 1.0 / D

    nc = bacc.Bacc("TRN2", target_bir_lowering=False, debug=False,
                   enable_asserts=False, num_devices=NCORES,
                   dynamic_dma_scratch_size=16384)

    def din(name, shape, dt):
        return nc.dram_tensor(name, list(shape), dt, kind="ExternalInput").ap()

    t_agtsT = din("agtsT", (P, NPC), bft)
    t_res = din("agts_res", (NPC, D), f32)
    t_ctx = din("ctx_bf", (N_CTX, D), bft)
    t_dT4 = din("dT4", (4, NT * P), bft)
    t_mask = din("maskh", (P, NT * P), bft)
    t_qidx = din("qidx", (P, NT * P // 16), i16)
    t_wlo = din("widx_lo", (P, N_LO * P // 16), i16)
    t_whi = din("widx_hi", (P, N_HI * P // 16), i16)
    wnames = ["w1_aug", "dist_w2", "q_w", "Wd", "Wq", "Wc", "ctx_w2",
              "agt_w", "lin_w"]
    t_w = {n: din(n, (4, D) if n == "w1_aug" else (D, DA), bft)
           for n in wnames}
    t_out = nc.dram_tensor("out", [NPC, D], f32, kind="ExternalOutput").ap()

    with tile.TileContext(nc) as tc, ExitStack() as ctx:
        const = ctx.enter_context(tc.tile_pool(name="const", bufs=1))
        big = ctx.enter_context(tc.tile_pool(name="big", bufs=1))
        dram = ctx.enter_context(tc.tile_pool(name="dram", bufs=1, space="DRAM"))
        sb = ctx.enter_context(tc.tile_pool(name="sb", bufs=4))
        gb = ctx.enter_context(tc.tile_pool(name="gb", bufs=3))
        ps = ctx.enter_context(tc.tile_pool(name="ps", bufs=2, space="PSUM"))

        # ---------- constants ----------
        ident_bf = const.tile([P, P], bft)
        idf = const.tile([P, P], f32)
        make_identity(nc, idf[:])
        nc.vector.tensor_copy(ident_bf[:], idf[:])
        eps_t = const.tile([P, 1], f32)
        nc.gpsimd.memset(eps_t[:], EPS)
        zeros_bf = const.tile([P, D], bft)
        nc.gpsimd.memset(zeros_bf[:], 0.0)
        junk = const.tile([P, D], bft)

        w_sb = {}
        for n in wnames:
            shp = [4, D] if n == "w1_aug" else [D, DA]
            w_sb[n] = const.tile(shp, bft, name=f"w_{n}")
            nc.sync.dma_start(w_sb[n][:], t_w[n][:])

        # big resident tensors
        agtsT = big.tile([P, NPC], bft)
        nc.sync.dma_start(agtsT[:], t_agtsT[:])
        qidx = big.tile([P, NT * P // 16], i16)
        nc.sync.dma_start(qidx[:], t_qidx[:])
        wlo = big.tile([P, N_LO * P // 16], i16)
        nc.sync.dma_start(wlo[:], t_wlo[:])
        whi = big.tile([P, N_HI * P // 16], i16)
        nc.sync.dma_start(whi[:], t_whi[:])

        Q2_dram = dram.tile([NPC, 256], bft)   # 129 used, padded for gather

        # ---------- phase 1: Q2_aug = (relu(gn(agts@q_w)) @ Wq_aug) ----------
        for g in range(NBLK // 4):        # 16 groups of 4 node chunks
            ps_q = ps.tile([P, 4, DA], f32, space="PSUM", tag="e",
                           padded_shape=(P, 4, 256))
            for c in range(4):
                j = g * 4 + c
                nc.tensor.matmul(ps_q[:, c, :], lhsT=agtsT[:, j * P:(j + 1) * P],
                                 rhs=w_sb["q_w"][:], start=True, stop=True)
            xq = sb.tile([P, 4, D], bft, tag="x1", bufs=8)
            nc.scalar.copy(xq[:], ps_q[:, :, 0:D])
            mq = sb.tile([P, 4, 1], f32, tag="m1", bufs=16)
            nc.vector.tensor_copy(mq[:], ps_q[:, :, D:DA])
            nv = sb.tile([P, 4], f32, tag="nv", bufs=16)
            for c in range(4):
                nc.vector.scalar_tensor_tensor(
                    out=junk[:], in0=xq[:, c, :], scalar=mq[:, c, :],
                    in1=xq[:, c, :], op0=ALU.subtract, op1=ALU.mult,
                    accum_out=nv[:, c:c + 1])
            sdq = sb.tile([P, 4], f32, tag="sd", bufs=16)
            nc.scalar.activation(sdq[:], nv[:], AF.Sqrt,
                                 bias=eps_t[:], scale=INV_D)
            rq = sb.tile([P, 4], f32, tag="rq", bufs=16)
            nc.vector.reciprocal(rq[:], sdq[:])
            qn = sb.tile([P, 4, D], bft, tag="qn", bufs=2)
            for c in range(4):
                nc.vector.tensor_scalar(
                    out=qn[:, c, :], in0=xq[:, c, :],
                    scalar1=mq[:, c, :], scalar2=rq[:, c:c + 1],
                    op0=ALU.subtract, op1=ALU.mult)
            ps_t = ps.tile([P, 4, D], bft, space="PSUM", tag="t")
            for c in range(4):
                nc.tensor.transpose(ps_t[:, c, :], qn[:, c, :], ident_bf[:])
            qnT = sb.tile([P, 4, D], bft, tag="qnT", bufs=2)
            nc.vector.tensor_scalar_max(qnT[:], ps_t[:], 0.0)
            ps_q2 = ps.tile([P, 4, DA], f32, space="PSUM", tag="e",
                            padded_shape=(P, 4, 256))
            for c in range(4):
                nc.tensor.matmul(ps_q2[:, c, :], lhsT=qnT[:, c, :],
                                 rhs=w_sb["Wq"][:], start=True, stop=True)
            q2sb = sb.tile([P, 4, DA], bft, tag="q2sb", bufs=2)
            nc.scalar.copy(q2sb[:], ps_q2[:])
            nc.sync.dma_start(
                Q2_dram[g * 512:(g + 1) * 512, 0:DA].rearrange(
                    "(c p) f -> p c f", p=P),
                q2sb[:])

        # ---------- phase 2: edge pipeline ----------
        lo_ctr = 0
        hi_ctr = 0
        tmeta = []                    # (blk, pos, half, slice_in_half_stream)
        for t in range(NT):
            b, pos = divmod(t, T_B)
            if pos < T_LO:
                tmeta.append((b, pos, 0, lo_ctr)); lo_ctr += 1
            else:
                tmeta.append((b, pos, 1, hi_ctr)); hi_ctr += 1

        qa_bufs = {}
        mask_bufs = {}
        lo_bufs = {}
        hi_bufs = {}
        dt_bufs = {}

        def issue_dt(gi):
            nt = min(G_TILES, NT - gi * G_TILES)
            buf = gb.tile([4, G_TILES * P], bft, tag="dt4", bufs=2)
            nc.sync.dma_start(buf[:, :nt * P],
                              t_dT4[:, gi * G_TILES * P:(gi * G_TILES + nt) * P])
            dt_bufs[gi] = buf

        def issue_qa(gi):
            nt = min(G_TILES, NT - gi * G_TILES)
            buf = gb.tile([P, G_TILES, 256], bft, tag="qa", bufs=2)
            nc.gpsimd.dma_gather(
                out_ap=buf[:, :nt, :], in_ap=Q2_dram[:],
                idxs_ap=qidx[:, gi * G_TILES * 8:(gi * G_TILES + nt) * 8],
                num_idxs=nt * P, num_idxs_reg=nt * P, elem_size=256,
                single_packet=False)
            qa_bufs[gi] = buf

        def issue_w(gi, half):
            n_str, src, idxt, bufs, tag = (
                (N_LO, t_ctx[:CTX_HALF, :], wlo, lo_bufs, "clo") if half == 0
                else (N_HI, t_ctx[CTX_HALF:, :], whi, hi_bufs, "chi"))
            nt = min(G_TILES, n_str - gi * G_TILES)
            buf = gb.tile([P, 1, G_TILES * P], bft, tag=tag, bufs=2)
            nc.gpsimd.dma_gather(
                out_ap=buf[:, :, :nt * P], in_ap=src,
                idxs_ap=idxt[:, gi * G_TILES * 8:(gi * G_TILES + nt) * 8],
                num_idxs=nt * P, num_idxs_reg=nt * P, elem_size=D,
                transpose=True, single_packet=False)
            bufs[gi] = buf

        assert T_LO % 4 == 0 and T_HI % 4 == 0, (T_LO, T_HI)
        assert G_TILES % 4 == 0
        assert T_B % 4 == 0

        # block PSUM state (allocated lazily per block)
        bps_g = {}
        bps_out = {}
        fin_x = {}                    # finale group SBUF staging [P,4,D] bf16
        fin_m = {}                    # finale group mean cols [P,4] f32

        def st_stages(s):
            """Edge pipeline for super-tile s (4 tiles in one node block)."""
            tiles = [4 * s + c for c in range(4)]
            b = tmeta[tiles[0]][0]
            assert all(tmeta[t][0] == b for t in tiles)
            first_st = (s % ST_PER_BLK) == 0
            last_st = (s % ST_PER_BLK) == ST_PER_BLK - 1
            gi0, off0 = divmod(4 * s, G_TILES)
            # stage 0: gathers / dT / mask loads
            if gi0 not in qa_bufs:
                issue_qa(gi0)
            if gi0 not in dt_bufs:
                issue_dt(gi0)
            for t in tiles:
                _b, _pos, half, si = tmeta[t]
                gi = si // G_TILES
                if half == 0 and gi not in lo_bufs:
                    issue_w(gi, 0)
                if half == 1 and gi not in hi_bufs:
                    issue_w(gi, 1)
            mgi = s // 4
            if mgi not in mask_bufs:
                mb_ = gb.tile([P, 16, D], bft, tag="mask", bufs=3, name="mb_")
                nc.sync.dma_start(
                    mb_[:], t_mask[:, mgi * 16 * P:(mgi + 1) * 16 * P].rearrange(
                        "p (c e) -> p c e", c=16))
                mask_bufs[mgi] = mb_
            mbuf = mask_bufs[mgi][:, 4 * (s % 4):4 * (s % 4) + 4, :]
            yield
            # stage 1: L1: y1T [f, 512e] = w1_aug.T @ dT4
            ps_y1 = ps.tile([P, 4 * D], f32, space="PSUM", tag="t",
                            padded_shape=(P, 512))
            nc.tensor.matmul(ps_y1[:], lhsT=w_sb["w1_aug"][:],
                             rhs=dt_bufs[gi0][:, off0 * P:(off0 + 4) * P],
                             start=True, stop=True)
            yield
            # stage 2: relu -> r1T [f, 4e] bf16 (ACT)
            r1T = sb.tile([P, 4 * D], bft, tag="r1T", bufs=6)
            nc.scalar.activation(r1T[:], ps_y1[:], AF.Relu)
            yield
            # stage 3: L2 -> ps2 [e, 4, 129] f32 (mean col via aug)
            ps2 = ps.tile([P, 4, DA], f32, space="PSUM", tag="e",
                          padded_shape=(P, 4, 256))
            for c in range(4):
                nc.tensor.matmul(ps2[:, c, :], lhsT=r1T[:, c * D:(c + 1) * D],
                                 rhs=w_sb["dist_w2"][:], start=True, stop=True)
            yield
            # stage 4: GN1 move (ACT) + mean col to SBUF (frees ps2)
            x1 = sb.tile([P, 4, D], bft, tag="x1", bufs=8)
            nc.scalar.copy(x1[:], ps2[:, :, 0:D])
            m1 = sb.tile([P, 4, 1], f32, tag="m1", bufs=16)
            nc.vector.tensor_copy(m1[:], ps2[:, :, D:DA])
            yield
            # stage 5: GN1 var via STT accum (Pool; SBUF-only operands)
            nv1 = sb.tile([P, 4], f32, tag="nv", bufs=16)
            for c in range(4):
                nc.vector.scalar_tensor_tensor(
                    out=junk[:], in0=x1[:, c, :], scalar=m1[:, c, :],
                    in1=x1[:, c, :], op0=ALU.subtract, op1=ALU.mult,
                    accum_out=nv1[:, c:c + 1])
            yield
            # stage 6: rinv1 (ACT)
            sd1 = sb.tile([P, 4], f32, tag="sd", bufs=16)
            nc.scalar.activation(sd1[:], nv1[:], AF.Sqrt,
                                 bias=eps_t[:], scale=INV_D)
            r1 = sb.tile([P, 4], f32, tag="r1", bufs=16)
            nc.vector.reciprocal(r1[:], sd1[:])
            yield
            # stage 7: GN1 apply (sub mean, mult rinv) -> h' (no relu yet)
            h = sb.tile([P, 4, D], bft, tag="h", bufs=6)
            for c in range(4):
                nc.gpsimd.tensor_scalar(
                    out=h[:, c, :], in0=x1[:, c, :],
                    scalar1=m1[:, c, :], scalar2=r1[:, c:c + 1],
                    op0=ALU.subtract, op1=ALU.mult)
            yield
            # stage 8: T(h')
            psT2 = ps.tile([P, 4, D], bft, space="PSUM", tag="t")
            for c in range(4):
                nc.tensor.transpose(psT2[:, c, :], h[:, c, :], ident_bf[:])
            yield
            # stage 9: hT = relu(psT2) (fused relu in copy, DVE 2x)
            hT = sb.tile([P, 4, D], bft, tag="hT", bufs=6)
            nc.vector.tensor_scalar_max(hT[:], psT2[:], 0.0)
            yield
            # stage 10: C1 = h@Wd_aug + cT.T@Wc_aug + Q2_aug[hi]
            ps3 = ps.tile([P, 4, DA], f32, space="PSUM", tag="e",
                          padded_shape=(P, 4, 256))
            for c, t in enumerate(tiles):
                _b, _pos, half, si = tmeta[t]
                gi, off = divmod(si, G_TILES)
                cbuf = lo_bufs[gi] if half == 0 else hi_bufs[gi]
                qgi, qoff = divmod(t, G_TILES)
                nc.tensor.matmul(ps3[:, c, :], lhsT=hT[:, c, :],
                                 rhs=w_sb["Wd"][:], start=True, stop=False)
                nc.tensor.matmul(ps3[:, c, :],
                                 lhsT=cbuf[:, 0, off * P:(off + 1) * P],
                                 rhs=w_sb["Wc"][:], start=False, stop=False)
                nc.tensor.matmul(ps3[:, c, :], lhsT=ident_bf[:],
                                 rhs=qa_bufs[qgi][:, qoff, 0:DA],
                                 start=False, stop=True)
            yield
            # stage 11: GN2 move (ACT) + mean col to SBUF (frees ps3)
            x2 = sb.tile([P, 4, D], bft, tag="x2", bufs=8)
            nc.scalar.copy(x2[:], ps3[:, :, 0:D])
            m2 = sb.tile([P, 4, 1], f32, tag="m1", bufs=16)
            nc.vector.tensor_copy(m2[:], ps3[:, :, D:DA])
            yield
            # stage 12: GN2 var (DVE)
            nv2 = sb.tile([P, 4], f32, tag="nv", bufs=16)
            for c in range(4):
                nc.vector.scalar_tensor_tensor(
                    out=junk[:], in0=x2[:, c, :], scalar=m2[:, c, :],
                    in1=x2[:, c, :], op0=ALU.subtract, op1=ALU.mult,
                    accum_out=nv2[:, c:c + 1])
            yield
            # stage 13: rinv2 (ACT)
            sd2 = sb.tile([P, 4], f32, tag="sd", bufs=16)
            nc.scalar.activation(sd2[:], nv2[:], AF.Sqrt,
                                 bias=eps_t[:], scale=INV_D)
            r2 = sb.tile([P, 4], f32, tag="r1", bufs=16)
            nc.vector.reciprocal(r2[:], sd2[:])
            yield
            # stage 14: g' = (x2 - m2) * rinv2 (TSP 4x)
            gp = sb.tile([P, 4, D], bft, tag="gp", bufs=6)
            for c in range(4):
                nc.gpsimd.tensor_scalar(
                    out=gp[:, c, :], in0=x2[:, c, :],
                    scalar1=m2[:, c, :], scalar2=r2[:, c:c + 1],
                    op0=ALU.subtract, op1=ALU.mult)
            yield
            # stage 15: g = relu(g') wide (DVE 4x)
            gt = sb.tile([P, 4, D], bft, tag="gt", bufs=6)
            nc.vector.tensor_scalar_max(gt[:], gp[:], 0.0)
            yield
            # stage 16: scatter g into block PSUM
            if first_st:
                if b % 2 == 0:
                    bo = ps.tile([P, 2, DA], f32, space="PSUM", tag="bo",
                                 bufs=1, padded_shape=(P, 2, 256))
                    bps_out[b] = bo
                else:
                    bps_out[b] = bps_out[b - 1]
                bg = ps.tile([P, D], f32, space="PSUM", tag="bg", bufs=1)
                bps_g[b] = bg
            for c in range(4):
                nc.tensor.matmul(bps_g[b][:], lhsT=gt[:, c, :],
                                 rhs=mbuf[:, c, :],
                                 start=(first_st and c == 0),
                                 stop=(last_st and c == 3))
            yield
            if not last_st:
                return
            # stage 17: block finish: bgT (already [f, n]) -> bf16 SBUF
            bgT = sb.tile([P, D], bft, tag="bgT")
            nc.scalar.copy(bgT[:], bps_g[b][:])
            yield
            # stage 18: block out = A1 + bgT.T@ctx_w2_aug (adjacent group)
            nc.tensor.matmul(bps_out[b][:, b % 2, :],
                             lhsT=agtsT[:, b * P:(b + 1) * P],
                             rhs=w_sb["agt_w"][:], start=True, stop=False)
            nc.tensor.matmul(bps_out[b][:, b % 2, :], lhsT=bgT[:],
                             rhs=w_sb["ctx_w2"][:], start=False, stop=True)
            yield
            # stage 21: stage block result to SBUF for finale
            g4 = b // 4
            if b % 4 == 0:
                fin_x[g4] = sb.tile([P, 4, D], bft, tag="finx", bufs=3,
                                    name="fin_x")
                fin_m[g4] = sb.tile([P, 4], f32, tag="finm", bufs=3,
                                    name="fin_m")
            nc.scalar.copy(fin_x[g4][:, b % 4, :], bps_out[b][:, b % 2, 0:D])
            nc.vector.tensor_copy(fin_m[g4][:, b % 4:b % 4 + 1],
                                  bps_out[b][:, b % 2, D:DA])
            yield

        def finale_stages(g4):
            """Node finale for 4 blocks: relu(gn(x)) @ lin_w, gn, +res, relu."""
            xg = fin_x[g4]
            mg = fin_m[g4]
            # norm-GN: rinv cancels into next GN; apply = relu(x - m)
            o1 = sb.tile([P, 4, D], bft, tag="o1", bufs=2)
            for c in range(4):
                nc.vector.tensor_scalar(
                    out=o1[:, c, :], in0=xg[:, c, :], scalar1=mg[:, c:c + 1],
                    scalar2=None, op0=ALU.subtract)
            yield
            ps_t = ps.tile([P, 4, D], bft, space="PSUM", tag="t")
            for c in range(4):
                nc.tensor.transpose(ps_t[:, c, :], o1[:, c, :], ident_bf[:])
            yield
            o1T = sb.tile([P, 4, D], bft, tag="o1T", bufs=2)
            nc.vector.tensor_scalar_max(o1T[:], ps_t[:], 0.0)
            yield
            ps_l = ps.tile([P, 4, DA], f32, space="PSUM", tag="e",
                           padded_shape=(P, 4, 256))
            for c in range(4):
                nc.tensor.matmul(ps_l[:, c, :], lhsT=o1T[:, c, :],
                                 rhs=w_sb["lin_w"][:], start=True, stop=True)
            yield
            xl = sb.tile([P, 4, D], bft, tag="xl", bufs=2)
            nc.scalar.copy(xl[:], ps_l[:, :, 0:D])
            ml = sb.tile([P, 4, 1], f32, tag="m1", bufs=16)
            nc.vector.tensor_copy(ml[:], ps_l[:, :, D:DA])
            yield
            nvl = sb.tile([P, 4], f32, tag="nv", bufs=16)
            for c in range(4):
                nc.vector.scalar_tensor_tensor(
                    out=junk[:], in0=xl[:, c, :], scalar=ml[:, c, :],
                    in1=xl[:, c, :], op0=ALU.subtract, op1=ALU.mult,
                    accum_out=nvl[:, c:c + 1])
            yield
            sdl = sb.tile([P, 4], f32, tag="sd", bufs=16)
            nc.scalar.activation(sdl[:], nvl[:], AF.Sqrt,
                                 bias=eps_t[:], scale=INV_D)
            rl = sb.tile([P, 4], f32, tag="r1", bufs=16)
            nc.vector.reciprocal(rl[:], sdl[:])
            yield
            o2 = sb.tile([P, 4, D], bft, tag="o2", bufs=2)
            for c in range(4):
                nc.vector.tensor_scalar(
                    out=o2[:, c, :], in0=xl[:, c, :],
                    scalar1=ml[:, c, :], scalar2=rl[:, c:c + 1],
                    op0=ALU.subtract, op1=ALU.mult)
            res_sb = sb.tile([P, 4, D], f32, tag="res_sb", bufs=2)
            nc.sync.dma_start(
                res_sb[:],
                t_res[g4 * 512:(g4 + 1) * 512, :].rearrange(
                    "(c p) f -> p c f", p=P))
            yield
            fin1 = sb.tile([P, 4, D], f32, tag="fin1", bufs=2)
            nc.gpsimd.tensor_tensor(out=fin1[:], in0=o2[:], in1=res_sb[:],
                                    op=ALU.add)
            yield
            fin = sb.tile([P, 4, D], f32, tag="fin", bufs=2)
            nc.scalar.activation(fin[:], fin1[:], AF.Relu)
            nc.sync.dma_start(
                t_out[g4 * 512:(g4 + 1) * 512, :].rearrange(
                    "(c p) f -> p c f", p=P),
                fin[:])
            yield

        # interleave STs (and finales when ready) to hide engine-hop latency
        ILV = 20

        def make_st(s):
            b = (4 * s) // T_B
            fg = None
            if (s % ST_PER_BLK) == ST_PER_BLK - 1 and b % 4 == 3:
                fg = b // 4     # this ST completes finale group fg
            return [st_stages(s), fg]

        work = []
        queue = []              # finale generators awaiting a slot
        next_st = 0
        while work or queue or next_st < NST:
            while len(work) < ILV and (queue or next_st < NST):
                if queue:
                    work.append([queue.pop(0), None])
                else:
                    work.append(make_st(next_st))
                    next_st += 1
            for item in list(work):
                try:
                    next(item[0])
                except StopIteration:
                    work.remove(item)
                    if item[1] is not None:
                        queue.append(finale_stages(item[1]))

    nc.compile()
    return nc


_cached = {}
_extra_run_kwargs = {}
_last_results = None


def run_traced(inputs):
    """Run once more with NTFF tracing; returns BassKernelResults."""
    global _extra_run_kwargs
    _extra_run_kwargs = dict(trace=True)
    try:
        kernel(**inputs)
    finally:
        _extra_run_kwargs = {}
    return _last_results


def kernel(agts, ctx, agt_ctrs, ctx_ctrs, hi, wi,
           dist_w1, dist_b1, dist_w2, dist_gw, dist_gb,
           q_w, q_gw, q_gb,
           ctx_w1, ctx_gw, ctx_gb, ctx_w2,
           agt_w, norm_w, norm_b,
           lin_w, lin_gw, lin_gb):
    for name, arr, val in (("dist_gw", dist_gw, 1), ("dist_gb", dist_gb, 0),
                           ("q_gw", q_gw, 1), ("q_gb", q_gb, 0),
                           ("ctx_gw", ctx_gw, 1), ("ctx_gb", ctx_gb, 0),
                           ("norm_w", norm_w, 1), ("norm_b", norm_b, 0),
                           ("lin_gw", lin_gw, 1), ("lin_gb", lin_gb, 0)):
        assert np.allclose(np.asarray(arr), val), f"{name} must be trivial"

    def aug(w):
        w = np.asarray(w, np.float32)
        return np.concatenate([w, w.sum(1, keepdims=True) / D], 1).astype(bf16)

    ctx_w1 = np.asarray(ctx_w1, np.float32)
    w1 = np.asarray(dist_w1, np.float32)
    b1 = np.asarray(dist_b1, np.float32)
    w1_aug = np.zeros((4, D), np.float32)
    w1_aug[0:2] = w1
    w1_aug[2] = b1
    weights = dict(
        w1_aug=w1_aug.astype(bf16),
        dist_w2=aug(dist_w2),
        q_w=aug(q_w),
        Wd=aug(ctx_w1[0:D]),
        Wq=aug(ctx_w1[D:2 * D]),
        Wc=aug(ctx_w1[2 * D:3 * D]),
        ctx_w2=aug(ctx_w2),
        agt_w=aug(agt_w),
        lin_w=aug(lin_w),
    )

    cores, shared, T_LO, T_HI = _host_prep(agts, ctx, agt_ctrs, ctx_ctrs,
                                           hi, wi, weights)
    key = (T_LO, T_HI)
    if key not in _cached:
        _cached[key] = _build_program(T_LO, T_HI)
    nc = _cached[key]

    in_maps = []
    for k in range(NCORES):
        m = dict(cores[k])
        m.update(shared)
        in_maps.append(m)

    res = run_bass_kernel_spmd(nc, in_maps, core_ids=list(range(NCORES)),
                               **_extra_run_kwargs)
    globals()["_last_results"] = res
    out = np.concatenate([res.results[k]["out"] for k in range(NCORES)], axis=0)
    return out.astype(np.float32)


if __name__ == "__main__":
    pass


# revision 62
# speedup vs baseline: 1.0318x; 1.0318x over previous
"""Trainium2 Bass kernel for nn_Att_23313082483285 (GNN message passing).

Strategy: partition agent nodes across 8 cores (8192 each). Host routes each
edge to the core owning its destination agent (hi), groups edges by 128-node
block, splits each block's edges by wi < 32768 (dma_gather idx is int16), and
pads each (block, half) to a fixed tile count so all cores run one SPMD
program.

v2 redesign (vs baseline): GroupNorm means come from an augmented 129th
weight column (mean is linear in the matmul output); variance via one fused
scalar_tensor_tensor (x-m)*x with accum_out per tile (4x DVE mode on bf16
SBUF); GN apply via tensor_scalar with two per-partition scalar pointers
(sub mean, mult rinv) at 4x; ReLU folded into the post-transpose PSUM->SBUF
copy or the apply's max op. One-hot scatter masks are host-precomputed and
DMA'd. The per-edge @ctx_w2 matmul is reassociated past the scatter-sum:
scatter relu'd g directly per tile, apply ctx_w2 once per 128-node block.
agts@agt_w accumulates directly into the block PSUM. The GroupNorm before
lin_w needs no variance at all (per-row scale invariance of the next GN).
"""
import sys
sys.path.insert(0, '/opt/trn_rl_repo')

import numpy as np
import ml_dtypes
from contextlib import ExitStack

from concourse import bass, mybir, tile
import concourse.bacc as bacc
from concourse.bass_utils import run_bass_kernel_spmd
from concourse.masks import make_identity

bf16 = ml_dtypes.bfloat16
P = 128
N_AGT = 65536
N_CTX = 65536
E = 400000
D = 128
DA = 129                       # D + mean column
EPS = 1e-5
NCORES = 8
NPC = N_AGT // NCORES          # 8192 nodes per core
NBLK = NPC // P                # 64 blocks per core
CTX_HALF = 32768
G_TILES = 32                   # tiles per dma_gather op (4096 idxs)

f32 = mybir.dt.float32
bft = mybir.dt.bfloat16
i16 = mybir.dt.int16
AF = mybir.ActivationFunctionType
ALU = mybir.AluOpType


def _wrap16(flat_idx):
    """dma_gather idx layout: [16, n/16] with idx[c,k]=flat[k*16+c], tiled x8."""
    w = flat_idx.reshape(-1, 16).T.astype(np.int16)
    return np.ascontiguousarray(np.tile(w, (8, 1)))


def _host_prep(agts, ctx, agt_ctrs, ctx_ctrs, hi, wi, weights):
    """Route/pad edges per core; build all per-core device input arrays."""
    hi = np.asarray(hi).astype(np.int64)
    wi = np.asarray(wi).astype(np.int64)
    agts = np.asarray(agts, dtype=np.float32)
    ctx = np.asarray(ctx, dtype=np.float32)
    d_all = (np.asarray(agt_ctrs, np.float32)[hi]
             - np.asarray(ctx_ctrs, np.float32)[wi])          # [E, 2]

    blk_global = hi // P          # 0..511
    lidx_all = hi % P
    is_hi = wi >= CTX_HALF

    # per (global block, half) edge lists
    lists = [[[] for _ in range(2)] for _ in range(N_AGT // P)]
    order = np.argsort(blk_global * 2 + is_hi, kind='stable')
    for e in order:
        lists[blk_global[e]][1 if is_hi[e] else 0].append(e)

    t_lo = max(max((len(l[0]) for l in lists), default=0), 1)
    t_hi = max(max((len(l[1]) for l in lists), default=0), 1)
    T_LO = -(-(-(-t_lo // P)) // 4) * 4   # round tiles up to multiple of 4
    T_HI = -(-(-(-t_hi // P)) // 4) * 4
    T_B = T_LO + T_HI
    NT = NBLK * T_B               # tiles per core
    EPAD = NT * P

    cores = []
    for k in range(NCORES):
        dT4 = np.zeros((4, EPAD), np.float32)
        mask = np.zeros((P, NT * P), np.float32)   # one-hot scatter masks
        qflat = np.zeros(EPAD, np.int64)
        lo_flat = np.zeros(NBLK * T_LO * P, np.int64)
        hi_flat = np.zeros(NBLK * T_HI * P, np.int64)

        for b in range(NBLK):
            gb = k * NBLK + b
            for half, (TH, flat) in enumerate(
                    ((T_LO, lo_flat), (T_HI, hi_flat))):
                edges = lists[gb][half]
                assert len(edges) <= TH * P, (
                    f"block overflow core {k} blk {b} half {half}: "
                    f"{len(edges)} > {TH * P}")
                for j, e in enumerate(edges):
                    tl = j // P          # tile within this half
                    p = j % P
                    t = b * T_B + (0 if half == 0 else T_LO) + tl  # global tile
                    col = t * P + p
                    dT4[0, col] = d_all[e, 0]
                    dT4[1, col] = d_all[e, 1]
                    dT4[2, col] = 1.0
                    mask[p, t * P + lidx_all[e]] = 1.0
                    qflat[col] = lidx_all[e] + (blk_global[e] % NBLK) * P
                    si = b * TH * P + tl * P + p   # slot in half-stream
                    flat[si] = wi[e] if half == 0 else wi[e] - CTX_HALF

        cores.append(dict(
            agtsT=np.ascontiguousarray(agts[k * NPC:(k + 1) * NPC].T.astype(bf16)),
            agts_res=np.ascontiguousarray(agts[k * NPC:(k + 1) * NPC]),
            dT4=dT4.astype(bf16),
            maskh=mask.astype(bf16),
            qidx=_wrap16(qflat),
            widx_lo=_wrap16(lo_flat),
            widx_hi=_wrap16(hi_flat),
        ))

    shared = dict(ctx_bf=np.ascontiguousarray(ctx.astype(bf16)), **weights)
    return cores, shared, T_LO, T_HI


def _build_program(T_LO, T_HI):
    T_B = T_LO + T_HI
    NT = NBLK * T_B
    NST = NT // 4                 # super-tiles of 4 tiles
    N_LO = NBLK * T_LO            # lo tiles per core
    N_HI = NBLK * T_HI
    ST_PER_BLK = T_B // 4         # STs per block (2 when T_B=8)
    INV_D = 1.0 / D

    nc = bacc.Bacc("TRN2", target_bir_lowering=False, debug=False,
                   enable_asserts=False, num_devices=NCORES,
                   dynamic_dma_scratch_size=16384)

    def din(name, shape, dt):
        return nc.dram_tensor(name, list(shape), dt, kind="ExternalInput").ap()

    t_agtsT = din("agtsT", (P, NPC), bft)
    t_res = din("agts_res", (NPC, D), f32)
    t_ctx = din("ctx_bf", (N_CTX, D), bft)
    t_dT4 = din("dT4", (4, NT * P), bft)
    t_mask = din("maskh", (P, NT * P), bft)
    t_qidx = din("qidx", (P, NT * P // 16), i16)
    t_wlo = din("widx_lo", (P, N_LO * P // 16), i16)
    t_whi = din("widx_hi", (P, N_HI * P // 16), i16)
    wnames = ["w1_aug", "dist_w2", "q_w", "Wd", "Wq", "Wc", "ctx_w2",
              "agt_w", "lin_w"]
    t_w = {n: din(n, (4, D) if n == "w1_aug" else (D, DA), bft)
           for n in wnames}
    t_out = nc.dram_tensor("out", [NPC, D], f32, kind="ExternalOutput").ap()

    with tile.TileContext(nc) as tc, ExitStack() as ctx:
        const = ctx.enter_context(tc.tile_pool(name="const", bufs=1))
        big = ctx.enter_context(tc.tile_pool(name="big", bufs=1))
        dram = ctx.enter_context(tc.tile_pool(name="dram", bufs=1, space="DRAM"))
        sb = ctx.enter_context(tc.tile_pool(name="sb", bufs=4))
        gb = ctx.enter_context(tc.tile_pool(name="gb", bufs=3))
        ps = ctx.enter_context(tc.tile_pool(name="ps", bufs=2, space="PSUM"))

        # ---------- constants ----------
        ident_bf = const.tile([P, P], bft)
        idf = const.tile([P, P], f32)
        make_identity(nc, idf[:])
        nc.vector.tensor_copy(ident_bf[:], idf[:])
        eps_t = const.tile([P, 1], f32)
        nc.gpsimd.memset(eps_t[:], EPS)
        junk = const.tile([P, D], bft)

        w_sb = {}
        for n in wnames:
            shp = [4, D] if n == "w1_aug" else [D, DA]
            w_sb[n] = const.tile(shp, bft, name=f"w_{n}")
            nc.sync.dma_start(w_sb[n][:], t_w[n][:])

        # big resident tensors
        agtsT = big.tile([P, NPC], bft)
        nc.sync.dma_start(agtsT[:], t_agtsT[:])
        qidx = big.tile([P, NT * P // 16], i16)
        nc.sync.dma_start(qidx[:], t_qidx[:])
        wlo = big.tile([P, N_LO * P // 16], i16)
        nc.sync.dma_start(wlo[:], t_wlo[:])
        whi = big.tile([P, N_HI * P // 16], i16)
        nc.sync.dma_start(whi[:], t_whi[:])

        Q2_dram = dram.tile([NPC, 256], bft)   # 129 used, padded for gather

        # ---------- phase 1: Q2_aug = (relu(gn(agts@q_w)) @ Wq_aug) ----------
        # emitted as generators interleaved with the edge pipeline
        def q_stages(g):
            ps_q = ps.tile([P, 4, DA], f32, space="PSUM", tag="e",
                           padded_shape=(P, 4, 256))
            for c in range(4):
                j = g * 4 + c
                nc.tensor.matmul(ps_q[:, c, :], lhsT=agtsT[:, j * P:(j + 1) * P],
                                 rhs=w_sb["q_w"][:], start=True, stop=True)
            yield
            xq = sb.tile([P, 4, D], bft, tag="qn", bufs=3)
            nc.scalar.copy(xq[:], ps_q[:, :, 0:D])
            mq = sb.tile([P, 4, 1], f32, tag="m1", bufs=16)
            nc.vector.tensor_copy(mq[:], ps_q[:, :, D:DA])
            yield
            nv = sb.tile([P, 4], f32, tag="nv", bufs=16)
            for c in range(4):
                nc.vector.scalar_tensor_tensor(
                    out=junk[:], in0=xq[:, c, :], scalar=mq[:, c, :],
                    in1=xq[:, c, :], op0=ALU.subtract, op1=ALU.mult,
                    accum_out=nv[:, c:c + 1])
            yield
            sdq = sb.tile([P, 4], f32, tag="sd", bufs=16)
            nc.scalar.activation(sdq[:], nv[:], AF.Sqrt,
                                 bias=eps_t[:], scale=INV_D)
            rq = sb.tile([P, 4], f32, tag="rq", bufs=16)
            nc.vector.reciprocal(rq[:], sdq[:])
            yield
            qn = sb.tile([P, 4, D], bft, tag="qn", bufs=3)
            for c in range(4):
                nc.vector.tensor_scalar(
                    out=qn[:, c, :], in0=xq[:, c, :],
                    scalar1=mq[:, c, :], scalar2=rq[:, c:c + 1],
                    op0=ALU.subtract, op1=ALU.mult)
            yield
            ps_t = ps.tile([P, 4, D], bft, space="PSUM", tag="t")
            for c in range(4):
                nc.tensor.transpose(ps_t[:, c, :], qn[:, c, :], ident_bf[:])
            yield
            qnT = sb.tile([P, 4, D], bft, tag="qnT", bufs=2)
            nc.vector.tensor_scalar_max(qnT[:], ps_t[:], 0.0)
            yield
            ps_q2 = ps.tile([P, 4, DA], f32, space="PSUM", tag="e",
                            padded_shape=(P, 4, 256))
            for c in range(4):
                nc.tensor.matmul(ps_q2[:, c, :], lhsT=qnT[:, c, :],
                                 rhs=w_sb["Wq"][:], start=True, stop=True)
            yield
            q2sb = sb.tile([P, 4, DA], bft, tag="q2sb", bufs=2)
            nc.scalar.copy(q2sb[:], ps_q2[:])
            nc.sync.dma_start(
                Q2_dram[g * 512:(g + 1) * 512, 0:DA].rearrange(
                    "(c p) f -> p c f", p=P),
                q2sb[:])
            yield

        # ---------- phase 2: edge pipeline ----------
        lo_ctr = 0
        hi_ctr = 0
        tmeta = []                    # (blk, pos, half, slice_in_half_stream)
        for t in range(NT):
            b, pos = divmod(t, T_B)
            if pos < T_LO:
                tmeta.append((b, pos, 0, lo_ctr)); lo_ctr += 1
            else:
                tmeta.append((b, pos, 1, hi_ctr)); hi_ctr += 1

        qa_bufs = {}
        mask_bufs = {}
        lo_bufs = {}
        hi_bufs = {}
        dt_bufs = {}

        def issue_dt(gi):
            nt = min(G_TILES, NT - gi * G_TILES)
            buf = gb.tile([4, G_TILES * P], bft, tag="dt4", bufs=2)
            nc.sync.dma_start(buf[:, :nt * P],
                              t_dT4[:, gi * G_TILES * P:(gi * G_TILES + nt) * P])
            dt_bufs[gi] = buf

        def issue_qa(gi):
            nt = min(G_TILES, NT - gi * G_TILES)
            buf = gb.tile([P, G_TILES, 256], bft, tag="qa", bufs=2)
            nc.gpsimd.dma_gather(
                out_ap=buf[:, :nt, :], in_ap=Q2_dram[:],
                idxs_ap=qidx[:, gi * G_TILES * 8:(gi * G_TILES + nt) * 8],
                num_idxs=nt * P, num_idxs_reg=nt * P, elem_size=256,
                single_packet=False)
            qa_bufs[gi] = buf

        def issue_w(gi, half):
            n_str, src, idxt, bufs, tag = (
                (N_LO, t_ctx[:CTX_HALF, :], wlo, lo_bufs, "clo") if half == 0
                else (N_HI, t_ctx[CTX_HALF:, :], whi, hi_bufs, "chi"))
            nt = min(G_TILES, n_str - gi * G_TILES)
            buf = gb.tile([P, 1, G_TILES * P], bft, tag=tag, bufs=2)
            nc.gpsimd.dma_gather(
                out_ap=buf[:, :, :nt * P], in_ap=src,
                idxs_ap=idxt[:, gi * G_TILES * 8:(gi * G_TILES + nt) * 8],
                num_idxs=nt * P, num_idxs_reg=nt * P, elem_size=D,
                transpose=True, single_packet=False)
            bufs[gi] = buf

        assert T_LO % 4 == 0 and T_HI % 4 == 0, (T_LO, T_HI)
        assert G_TILES % 4 == 0
        assert T_B % 4 == 0

        # block PSUM state (allocated lazily per block)
        bps_g = {}
        bps_out = {}
        fin_x = {}                    # finale group SBUF staging [P,4,D] bf16
        fin_m = {}                    # finale group mean cols [P,4] f32

        def st_stages(s):
            """Edge pipeline for super-tile s (4 tiles in one node block)."""
            tiles = [4 * s + c for c in range(4)]
            b = tmeta[tiles[0]][0]
            assert all(tmeta[t][0] == b for t in tiles)
            first_st = (s % ST_PER_BLK) == 0
            last_st = (s % ST_PER_BLK) == ST_PER_BLK - 1
            gi0, off0 = divmod(4 * s, G_TILES)
            # stage 0: gathers / dT / mask loads
            if gi0 not in qa_bufs:
                issue_qa(gi0)
            if gi0 not in dt_bufs:
                issue_dt(gi0)
            for t in tiles:
                _b, _pos, half, si = tmeta[t]
                gi = si // G_TILES
                if half == 0 and gi not in lo_bufs:
                    issue_w(gi, 0)
                if half == 1 and gi not in hi_bufs:
                    issue_w(gi, 1)
            mgi = s // 4
            if mgi not in mask_bufs:
                mb_ = gb.tile([P, 16, D], bft, tag="mask", bufs=3, name="mb_")
                nc.sync.dma_start(
                    mb_[:], t_mask[:, mgi * 16 * P:(mgi + 1) * 16 * P].rearrange(
                        "p (c e) -> p c e", c=16))
                mask_bufs[mgi] = mb_
            mbuf = mask_bufs[mgi][:, 4 * (s % 4):4 * (s % 4) + 4, :]
            yield
            # stage 1: L1: y1T [f, 512e] = w1_aug.T @ dT4
            ps_y1 = ps.tile([P, 4 * D], f32, space="PSUM", tag="t",
                            padded_shape=(P, 512))
            nc.tensor.matmul(ps_y1[:], lhsT=w_sb["w1_aug"][:],
                             rhs=dt_bufs[gi0][:, off0 * P:(off0 + 4) * P],
                             start=True, stop=True)
            yield
            # stage 2: relu -> r1T [f, 4e] bf16 (ACT)
            r1T = sb.tile([P, 4 * D], bft, tag="r1T", bufs=6)
            nc.scalar.activation(r1T[:], ps_y1[:], AF.Relu)
            yield
            # stage 3: L2 -> ps2 [e, 4, 129] f32 (mean col via aug)
            ps2 = ps.tile([P, 4, DA], f32, space="PSUM", tag="e",
                          padded_shape=(P, 4, 256))
            for c in range(4):
                nc.tensor.matmul(ps2[:, c, :], lhsT=r1T[:, c * D:(c + 1) * D],
                                 rhs=w_sb["dist_w2"][:], start=True, stop=True)
            yield
            # stage 4: GN1 move (ACT) + mean col to SBUF (frees ps2)
            x1 = sb.tile([P, 4, D], bft, tag="x1", bufs=8)
            nc.scalar.copy(x1[:], ps2[:, :, 0:D])
            m1 = sb.tile([P, 4, 1], f32, tag="m1", bufs=16)
            nc.vector.tensor_copy(m1[:], ps2[:, :, D:DA])
            yield
            # stage 5: GN1 var via STT accum (Pool; SBUF-only operands)
            nv1 = sb.tile([P, 4], f32, tag="nv", bufs=16)
            for c in range(4):
                nc.vector.scalar_tensor_tensor(
                    out=junk[:], in0=x1[:, c, :], scalar=m1[:, c, :],
                    in1=x1[:, c, :], op0=ALU.subtract, op1=ALU.mult,
                    accum_out=nv1[:, c:c + 1])
            yield
            # stage 6: rinv1 (ACT)
            sd1 = sb.tile([P, 4], f32, tag="sd", bufs=16)
            nc.scalar.activation(sd1[:], nv1[:], AF.Sqrt,
                                 bias=eps_t[:], scale=INV_D)
            r1 = sb.tile([P, 4], f32, tag="r1", bufs=16)
            nc.vector.reciprocal(r1[:], sd1[:])
            yield
            # stage 7: GN1 apply (sub mean, mult rinv) -> h' (no relu yet)
            h = sb.tile([P, 4, D], bft, tag="h", bufs=6)
            for c in range(4):
                nc.gpsimd.tensor_scalar(
                    out=h[:, c, :], in0=x1[:, c, :],
                    scalar1=m1[:, c, :], scalar2=r1[:, c:c + 1],
                    op0=ALU.subtract, op1=ALU.mult)
            yield
            # stage 8: T(h')
            psT2 = ps.tile([P, 4, D], bft, space="PSUM", tag="t")
            for c in range(4):
                nc.tensor.transpose(psT2[:, c, :], h[:, c, :], ident_bf[:])
            yield
            # stage 9: hT = relu(psT2) (fused relu in copy, DVE 2x)
            hT = sb.tile([P, 4, D], bft, tag="hT", bufs=6)
            nc.vector.tensor_scalar_max(hT[:], psT2[:], 0.0)
            yield
            # stage 10: C1 = h@Wd_aug + cT.T@Wc_aug + Q2_aug[hi]
            ps3 = ps.tile([P, 4, DA], f32, space="PSUM", tag="e",
                          padded_shape=(P, 4, 256))
            for c, t in enumerate(tiles):
                _b, _pos, half, si = tmeta[t]
                gi, off = divmod(si, G_TILES)
                cbuf = lo_bufs[gi] if half == 0 else hi_bufs[gi]
                qgi, qoff = divmod(t, G_TILES)
                nc.tensor.matmul(ps3[:, c, :], lhsT=hT[:, c, :],
                                 rhs=w_sb["Wd"][:], start=True, stop=False)
                nc.tensor.matmul(ps3[:, c, :],
                                 lhsT=cbuf[:, 0, off * P:(off + 1) * P],
                                 rhs=w_sb["Wc"][:], start=False, stop=False)
                nc.tensor.matmul(ps3[:, c, :], lhsT=ident_bf[:],
                                 rhs=qa_bufs[qgi][:, qoff, 0:DA],
                                 start=False, stop=True)
            yield
            # stage 11: GN2 move (ACT) + mean col to SBUF (frees ps3)
            x2 = sb.tile([P, 4, D], bft, tag="x2", bufs=8)
            nc.scalar.copy(x2[:], ps3[:, :, 0:D])
            m2 = sb.tile([P, 4, 1], f32, tag="m1", bufs=16)
            nc.vector.tensor_copy(m2[:], ps3[:, :, D:DA])
            yield
            # stage 12: GN2 var (DVE)
            nv2 = sb.tile([P, 4], f32, tag="nv", bufs=16)
            for c in range(4):
                nc.vector.scalar_tensor_tensor(
                    out=junk[:], in0=x2[:, c, :], scalar=m2[:, c, :],
                    in1=x2[:, c, :], op0=ALU.subtract, op1=ALU.mult,
                    accum_out=nv2[:, c:c + 1])
            yield
            # stage 13: rinv2 (ACT)
            sd2 = sb.tile([P, 4], f32, tag="sd", bufs=16)
            nc.scalar.activation(sd2[:], nv2[:], AF.Sqrt,
                                 bias=eps_t[:], scale=INV_D)
            r2 = sb.tile([P, 4], f32, tag="r1", bufs=16)
            nc.vector.reciprocal(r2[:], sd2[:])
            yield
            # stage 14: g' = (x2 - m2) * rinv2 (TSP 4x)
            gp = sb.tile([P, 4, D], bft, tag="gp", bufs=6)
            for c in range(4):
                nc.gpsimd.tensor_scalar(
                    out=gp[:, c, :], in0=x2[:, c, :],
                    scalar1=m2[:, c, :], scalar2=r2[:, c:c + 1],
                    op0=ALU.subtract, op1=ALU.mult)
            yield
            # stage 15: g = relu(g') wide (DVE 4x)
            gt = sb.tile([P, 4, D], bft, tag="gt", bufs=6)
            nc.vector.tensor_scalar_max(gt[:], gp[:], 0.0)
            yield
            # stage 16: scatter g into block PSUM
            if first_st:
                if b % 2 == 0:
                    bo = ps.tile([P, 2, DA], f32, space="PSUM", tag="bo",
                                 bufs=1, padded_shape=(P, 2, 256))
                    bps_out[b] = bo
                else:
                    bps_out[b] = bps_out[b - 1]
                bg = ps.tile([P, 2, D], f32, space="PSUM", tag="bg",
                             bufs=1, padded_shape=(P, 2, 256))
                bps_g[b] = bg
            sub = s % ST_PER_BLK
            for c in range(4):
                nc.tensor.matmul(bps_g[b][:, sub, :], lhsT=gt[:, c, :],
                                 rhs=mbuf[:, c, :],
                                 start=(c == 0), stop=(c == 3))
            yield
            if not last_st:
                return
            # stage 17: block finish: merge the two sub-accumulators
            bga = sb.tile([P, D], bft, tag="bgsb", bufs=2)
            nc.scalar.copy(bga[:], bps_g[b][:, 0, :])
            yield
            bgT = sb.tile([P, D], bft, tag="bgT", bufs=2)
            nc.vector.scalar_tensor_tensor(
                out=bgT[:], in0=bps_g[b][:, 1, :], scalar=0.0,
                in1=bga[:], op0=ALU.add, op1=ALU.add)
            yield
            # stage 18: block out = A1 + bgT.T@ctx_w2_aug (adjacent group)
            nc.tensor.matmul(bps_out[b][:, b % 2, :],
                             lhsT=agtsT[:, b * P:(b + 1) * P],
                             rhs=w_sb["agt_w"][:], start=True, stop=False)
            nc.tensor.matmul(bps_out[b][:, b % 2, :], lhsT=bgT[:],
                             rhs=w_sb["ctx_w2"][:], start=False, stop=True)
            yield
            # stage 21: stage block result to SBUF for finale
            g4 = b // 4
            if b % 4 == 0:
                fin_x[g4] = sb.tile([P, 4, D], bft, tag="finx", bufs=3,
                                    name="fin_x")
                fin_m[g4] = sb.tile([P, 4], f32, tag="finm", bufs=3,
                                    name="fin_m")
            nc.scalar.copy(fin_x[g4][:, b % 4, :], bps_out[b][:, b % 2, 0:D])
            nc.vector.tensor_copy(fin_m[g4][:, b % 4:b % 4 + 1],
                                  bps_out[b][:, b % 2, D:DA])
            yield

        def finale_stages(g4):
            """Node finale for 4 blocks: relu(gn(x)) @ lin_w, gn, +res, relu."""
            xg = fin_x[g4]
            mg = fin_m[g4]
            # norm-GN: rinv cancels into next GN; apply = relu(x - m)
            o1 = sb.tile([P, 4, D], bft, tag="o1", bufs=2)
            for c in range(4):
                nc.vector.tensor_scalar(
                    out=o1[:, c, :], in0=xg[:, c, :], scalar1=mg[:, c:c + 1],
                    scalar2=None, op0=ALU.subtract)
            yield
            ps_t = ps.tile([P, 4, D], bft, space="PSUM", tag="t")
            for c in range(4):
                nc.tensor.transpose(ps_t[:, c, :], o1[:, c, :], ident_bf[:])
            yield
            o1T = sb.tile([P, 4, D], bft, tag="o1T", bufs=2)
            nc.vector.tensor_scalar_max(o1T[:], ps_t[:], 0.0)
            yield
            ps_l = ps.tile([P, 4, DA], f32, space="PSUM", tag="e",
                           padded_shape=(P, 4, 256))
            for c in range(4):
                nc.tensor.matmul(ps_l[:, c, :], lhsT=o1T[:, c, :],
                                 rhs=w_sb["lin_w"][:], start=True, stop=True)
            yield
            xl = sb.tile([P, 4, D], bft, tag="xl", bufs=2)
            nc.scalar.copy(xl[:], ps_l[:, :, 0:D])
            ml = sb.tile([P, 4, 1], f32, tag="m1", bufs=16)
            nc.vector.tensor_copy(ml[:], ps_l[:, :, D:DA])
            yield
            nvl = sb.tile([P, 4], f32, tag="nv", bufs=16)
            for c in range(4):
                nc.vector.scalar_tensor_tensor(
                    out=junk[:], in0=xl[:, c, :], scalar=ml[:, c, :],
                    in1=xl[:, c, :], op0=ALU.subtract, op1=ALU.mult,
                    accum_out=nvl[:, c:c + 1])
            yield
            sdl = sb.tile([P, 4], f32, tag="sd", bufs=16)
            nc.scalar.activation(sdl[:], nvl[:], AF.Sqrt,
                                 bias=eps_t[:], scale=INV_D)
            rl = sb.tile([P, 4], f32, tag="r1", bufs=16)
            nc.vector.reciprocal(rl[:], sdl[:])
            yield
            o2 = sb.tile([P, 4, D], bft, tag="o2", bufs=2)
            for c in range(4):
                nc.vector.tensor_scalar(
                    out=o2[:, c, :], in0=xl[:, c, :],
                    scalar1=ml[:, c, :], scalar2=rl[:, c:c + 1],
                    op0=ALU.subtract, op1=ALU.mult)
            res_sb = sb.tile([P, 4, D], f32, tag="res_sb", bufs=2)
            nc.sync.dma_start(
                res_sb[:],
                t_res[g4 * 512:(g4 + 1) * 512, :].rearrange(
                    "(c p) f -> p c f", p=P))
            yield
            fin1 = sb.tile([P, 4, D], f32, tag="fin1", bufs=2)
            nc.gpsimd.tensor_tensor(out=fin1[:], in0=o2[:], in1=res_sb[:],
                                    op=ALU.add)
            yield
            fin = sb.tile([P, 4, D], f32, tag="fin", bufs=2)
            nc.scalar.activation(fin[:], fin1[:], AF.Relu)
            nc.sync.dma_start(
                t_out[g4 * 512:(g4 + 1) * 512, :].rearrange(
                    "(c p) f -> p c f", p=P),
                fin[:])
            yield

        # interleave STs (and finales when ready) to hide engine-hop latency
        ILV = 20

        def make_st(s):
            b = (4 * s) // T_B
            fg = None
            if (s % ST_PER_BLK) == ST_PER_BLK - 1 and b % 4 == 3:
                fg = b // 4     # this ST completes finale group fg
            return [st_stages(s), fg]

        # phase-1 groups 0..5 run serialized up front (their gathers fire
        # within the first super-tiles and would race); groups 6..15
        # interleave -- their gathers fire >=40 STs after the writes.
        for _g in range(5):
            for _ in q_stages(_g):
                pass
        work = []
        queue = []
        qgens = [q_stages(g) for g in range(5, NBLK // 4)]
        next_st = 0
        while work or queue or next_st < NST or qgens:
            while len(work) < ILV and (queue or next_st < NST or qgens):
                nq = sum(1 for it in work if it[1] == 'q')
                if queue:
                    work.append([queue.pop(0), None])
                elif qgens and nq < 2:
                    work.append([qgens.pop(0), 'q'])
                elif next_st < NST:
                    work.append(make_st(next_st))
                    next_st += 1
                else:
                    work.append([qgens.pop(0), 'q'])
            for item in list(work):
                try:
                    next(item[0])
                except StopIteration:
                    work.remove(item)
                    if item[1] is not None and item[1] != 'q':
                        queue.append(finale_stages(item[1]))

    nc.compile()
    return nc


_cached = {}
_extra_run_kwargs = {}
_last_results = None


def run_traced(inputs):
    """Run once more with NTFF tracing; returns BassKernelResults."""
    global _extra_run_kwargs
    _extra_run_kwargs = dict(trace=True)
    try:
        kernel(**inputs)
    finally:
        _extra_run_kwargs = {}
    return _last_results


def kernel(agts, ctx, agt_ctrs, ctx_ctrs, hi, wi,
           dist_w1, dist_b1, dist_w2, dist_gw, dist_gb,
           q_w, q_gw, q_gb,
           ctx_w1, ctx_gw, ctx_gb, ctx_w2,
           agt_w, norm_w, norm_b,
           lin_w, lin_gw, lin_gb):
    for name, arr, val in (("dist_gw", dist_gw, 1), ("dist_gb", dist_gb, 0),
                           ("q_gw", q_gw, 1), ("q_gb", q_gb, 0),
                           ("ctx_gw", ctx_gw, 1), ("ctx_gb", ctx_gb, 0),
                           ("norm_w", norm_w, 1), ("norm_b", norm_b, 0),
                           ("lin_gw", lin_gw, 1), ("lin_gb", lin_gb, 0)):
        assert np.allclose(np.asarray(arr), val), f"{name} must be trivial"

    def aug(w):
        w = np.asarray(w, np.float32)
        return np.concatenate([w, w.sum(1, keepdims=True) / D], 1).astype(bf16)

    ctx_w1 = np.asarray(ctx_w1, np.float32)
    w1 = np.asarray(dist_w1, np.float32)
    b1 = np.asarray(dist_b1, np.float32)
    w1_aug = np.zeros((4, D), np.float32)
    w1_aug[0:2] = w1
    w1_aug[2] = b1
    weights = dict(
        w1_aug=w1_aug.astype(bf16),
        dist_w2=aug(dist_w2),
        q_w=aug(q_w),
        Wd=aug(ctx_w1[0:D]),
        Wq=aug(ctx_w1[D:2 * D]),
        Wc=aug(ctx_w1[2 * D:3 * D]),
        ctx_w2=aug(ctx_w2),
        agt_w=aug(agt_w),
        lin_w=aug(lin_w),
    )

    cores, shared, T_LO, T_HI = _host_prep(agts, ctx, agt_ctrs, ctx_ctrs,
                                           hi, wi, weights)
    key = (T_LO, T_HI)
    if key not in _cached:
        _cached[key] = _build_program(T_LO, T_HI)
    nc = _cached[key]

    in_maps = []
    for k in range(NCORES):
        m = dict(cores[k])
        m.update(shared)
        in_maps.append(m)

    res = run_bass_kernel_spmd(nc, in_maps, core_ids=list(range(NCORES)),
                               **_extra_run_kwargs)
    globals()["_last_results"] = res
    out = np.concatenate([res.results[k]["out"] for k in range(NCORES)], axis=0)
    return out.astype(np.float32)


if __name__ == "__main__":
    pass
